# revision 1
# baseline (speedup 1.0000x reference)
"""Contrastive projection head loss on 8 Trainium2 NeuronCores (fp8 DoubleRow).

Reference computation (B=8192, E=1024, P=512):
    z_codon = relu(x[:, :E]) @ w + b          # [B, P]
    z_amino = relu(x[:, E:]) @ w + b          # [B, P]
    z  = concat([z_codon, z_amino], axis=1)   # [B, 2P]
    zn = z / max(||z||, 1e-8)
    s  = (zn @ zn.T);  s[i,i] = -9e15;  s /= 0.1
    nll_i = -s[i, (i - B/2) % B] + logsumexp(s[i, :])
    out = mean(nll)

v2 changes vs v1 (341.7us):
 - All PE math in fp8e4 (e4m3). Similarity + projection GEMMs use
   MatmulPerfMode.DoubleRow (two 128-feature k-tiles per instruction,
   0.5 cycles/row = 4x the measured bf16 rate). Shipped zn is scaled
   x16 (w is scaled x32) to stay in e4m3's normal range; the scales are
   folded into the exp scale (10/256) and the projection output scale.
 - x transposed on the PE in fp8 (1 cycle/row vs 2 for fp32), relu
   fused into the f32->fp8 cast on Vector.
 - AllGather payload halves (fp8): less exposed collective time.
 - Phase-2 loop restructured: local-block prelude (no AG dependency)
   runs while the collectives fly; remote blocks iterate dd-inner so
   five PSUM banks accumulate in parallel under shared stationary.
 - Column sums accumulate on Vector (jacc) instead of per-tile PE
   ones-matmuls; one PE matmul per (k,dd) at the end.
 - DMA spread across engine rings (sync/scalar/vector) to avoid the
   v1 head-of-line stall (normalize blocked behind AG-gated rhs load).
 - norm reciprocal via reciprocal_approx_fast (~5x faster).

Returns per-core partial sums [1, 8]; host sums and divides by B.
"""
import numpy as np

from concourse import bass, mybir, tile, bacc
from concourse.bass_utils import run_bass_kernel_spmd
from concourse.masks import make_identity

N_CORES = 8
B = 8192
E = 1024          # embedding size (per half)
P = 512           # projection size
D = 2 * P         # z feature dim = 1024
R = B // N_CORES  # rows per core = 1024
KT = D // 128     # feature sub-tiles = 8
MT = R // 128     # row sub-tiles per core = 8
INV_T = 10.0      # 1 / temperature
EPS = 1e-8
SC_Z = 16.0       # zn pre-scale before fp8 cast
SC_W = 32.0       # w pre-scale before fp8 cast
EXP_SC = INV_T / (SC_Z * SC_Z)   # activation scale recovering s/T

F32 = mybir.dt.float32
F32R = mybir.dt.float32r
BF16 = mybir.dt.bfloat16
FP8 = mybir.dt.float8e4
FP8E5 = mybir.dt.float8e5
AF = mybir.ActivationFunctionType
ALU = mybir.AluOpType
DR = mybir.MatmulPerfMode.DoubleRow

NSLOT = 10        # rowsum slots: 2 local (d=0) + 8 remote (d=1..4, k=0..1)

_cached = {}


def _build(no_collective=False):
    nc = bacc.Bacc("TRN2", target_bir_lowering=False, debug=False,
                   enable_asserts=False, num_devices=N_CORES)
    x_in = nc.dram_tensor("xs", [R, 2 * E], F32, kind="ExternalInput").ap()
    w_in = nc.dram_tensor("w", [E, P], F32, kind="ExternalInput").ap()
    b_in = nc.dram_tensor("b", [P], F32, kind="ExternalInput").ap()
    out = nc.dram_tensor("out", [1, MT], F32, kind="ExternalOutput").ap()

    with tile.TileContext(nc) as tc:
        with tc.tile_pool(name="const", bufs=1) as const, \
             tc.tile_pool(name="big", bufs=1) as big, \
             tc.tile_pool(name="small", bufs=1) as small, \
             tc.tile_pool(name="dram", bufs=1, space="DRAM") as dram:

            ident = const.tile([128, 128], F32)
            make_identity(nc, ident[:])
            idb = const.tile([128, 128], BF16)
            make_identity(nc, idb[:])
            ones_f = const.tile([128, 1], F32)
            nc.vector.memset(ones_f[:], 1.0)
            ones_r = const.tile([128, 1], F32R)
            nc.vector.tensor_copy(ones_r[:], ones_f[:])
            ones_b = const.tile([128, 1], BF16)
            nc.vector.tensor_copy(ones_b[:], ones_f[:])
            ones_e5 = const.tile([128, 1], FP8E5)
            nc.vector.memset(ones_e5[:], 1.0)
            b2 = const.tile([128, P // 128], F32)
            nc.sync.dma_start(b2[:], b_in.rearrange("(mt p) -> p mt", p=128))
            rn_bc = const.tile([128, R], F32)

            # w as [128, KT(=E/128), P] fp8, scaled x32 (staged inside
            # phase 1 so the 2MB load queues behind the first x rows)
            w8 = const.tile([128, E // 128, P], FP8)

            # znT8: zn x16, fp8, feature-major [128, KT, R]
            znT8 = big.tile([128, KT, R], FP8, tag="zn8")
            ag_in = [dram.tile([D, 512], FP8, name=f"ag_in{k}")
                     for k in range(2)]
            ag_out = [dram.tile([N_CORES * D, 512], FP8, name=f"ag_out{k}",
                                addr_space="Local" if no_collective else "Shared")
                      for k in range(2)]
            rn_dram = dram.tile([R], F32)
            cs_in = [dram.tile([3, 512], F32, name=f"cs_in{k}")
                     for k in range(2)]
            cs_out = [dram.tile([N_CORES * 3, 512], F32, name=f"cs_out{k}",
                                addr_space="Local" if no_collective else "Shared")
                      for k in range(2)]

            # ---- phase 1, pipelined over row-halves jh ----
            with tc.tile_pool(name="xrow", bufs=4) as xrowp, \
                 tc.tile_pool(name="wst", bufs=1) as wstp, \
                 tc.tile_pool(name="x8p", bufs=2) as x8p, \
                 tc.tile_pool(name="xTp", bufs=2) as xTp, \
                 tc.tile_pool(name="zTp", bufs=1) as zTp, \
                 tc.tile_pool(name="sqp", bufs=2) as sqp, \
                 tc.tile_pool(name="ps1", bufs=2, space="PSUM") as ps1:
                zT = zTp.tile([128, KT, R], F32, tag="zT")
                for jh in range(2):
                    # transpose rows of this half (both x halves) in fp8,
                    # relu fused into the cast
                    xT8 = xTp.tile([128, 2 * KT, 512], FP8, tag="xT",
                                   name=f"xT{jh}")
                    for r in range(4):
                        rg = jh * 4 + r
                        xrow = xrowp.tile([128, 2 * E], F32, tag="xrow")
                        nc.sync.dma_start(xrow[:, :E],
                                          x_in[rg * 128:(rg + 1) * 128, :E])
                        nc.scalar.dma_start(xrow[:, E:],
                                            x_in[rg * 128:(rg + 1) * 128, E:])
                        xbrow = x8p.tile([128, 2 * E], BF16, tag="x8")
                        nc.vector.tensor_scalar_max(xbrow[:], xrow[:], 0.0)
                        for cg in range(2 * E // 512):
                            pt = ps1.tile([128, 4, 128], BF16, tag="tp",
                                          bufs=3)
                            for q in range(4):
                                ct = cg * 4 + q
                                nc.tensor.transpose(
                                    pt[:, q, :],
                                    xbrow[:, ct * 128:(ct + 1) * 128],
                                    idb[:])
                            nc.vector.tensor_copy(
                                xT8[:, cg * 4:(cg + 1) * 4,
                                    r * 128:(r + 1) * 128],
                                pt[:])
                    if jh == 0:
                        wstage = wstp.tile([128, E // 128, P], F32,
                                           tag="wstage")
                        wsrc = w_in.rearrange("(kt p) q -> p kt q", p=128)
                        nc.sync.dma_start(wstage[:, :E // 256, :],
                                          wsrc[:, :E // 256, :])
                        nc.scalar.dma_start(wstage[:, E // 256:, :],
                                            wsrc[:, E // 256:, :])
                        nc.vector.tensor_scalar_mul(w8[:], wstage[:], SC_W)
                    # project this half with DoubleRow fp8
                    for h in range(2):
                        for m4 in range(P // 128):
                            pz = ps1.tile([128, 512], F32, tag="pz", bufs=2)
                            for t in range(4):
                                nc.tensor.matmul(
                                    pz[:],
                                    w8[:, 2 * t:2 * t + 2,
                                       m4 * 128:(m4 + 1) * 128],
                                    xT8[:, h * KT + 2 * t:h * KT + 2 * t + 2, :],
                                    start=(t == 0), stop=(t == 3),
                                    perf_mode=DR)
                            # z = pz/SC_W + b
                            nc.scalar.activation(
                                zT[:, h * 4 + m4, jh * 512:(jh + 1) * 512],
                                pz[:], AF.Identity,
                                bias=b2[:, m4:m4 + 1], scale=1.0 / SC_W)
                    # row inv-norms (x SC_Z) for this half
                    pn = ps1.tile([1, 512], F32, tag="pn", bufs=2,
                                  name=f"pn{jh}")
                    for kt in range(KT):
                        sq = sqp.tile([128, 512], F32R, tag="sq")
                        zsl = zT[:, kt, jh * 512:(jh + 1) * 512]
                        nc.vector.tensor_tensor(sq[:], zsl, zsl, ALU.mult)
                        nc.tensor.matmul(pn[:], ones_r[:], sq[:],
                                         start=(kt == 0), stop=(kt == KT - 1))
                    nrm = small.tile([1, 512], F32, tag="nrm", name=f"nrm{jh}")
                    # sqrt(sum/SC_Z^2) = norm/SC_Z
                    nc.scalar.activation(nrm[:], pn[:], AF.Sqrt,
                                         scale=1.0 / (SC_Z * SC_Z))
                    rn_strip = small.tile([1, 512], F32, tag="rns",
                                          name=f"rns{jh}")
                    nc.vector.reciprocal_approx_fast(rn_strip[:], nrm[:])
                    nc.scalar.dma_start(rn_dram[None, jh * 512:(jh + 1) * 512],
                                        rn_strip[:])
                    nc.scalar.dma_start(
                        rn_bc[:, jh * 512:(jh + 1) * 512],
                        rn_dram[None, jh * 512:(jh + 1) * 512]
                        .to_broadcast([128, 512]))
                    # normalize (x SC_Z) into fp8 and ship this half
                    for kt in range(KT):
                        nc.vector.tensor_tensor(
                            znT8[:, kt, jh * 512:(jh + 1) * 512],
                            zT[:, kt, jh * 512:(jh + 1) * 512],
                            rn_bc[:, jh * 512:(jh + 1) * 512], ALU.mult)
                    nc.scalar.dma_start(
                        ag_in[jh].rearrange("(kt p) j -> p kt j", p=128),
                        znT8[:, :, jh * 512:(jh + 1) * 512])
                    if no_collective:
                        for c in range(N_CORES):
                            nc.sync.dma_start(
                                ag_out[jh][c * D:(c + 1) * D, :], ag_in[jh][:])
                    else:
                        nc.gpsimd.collective_compute(
                            "AllGather", ALU.bypass,
                            replica_groups=[list(range(N_CORES))],
                            ins=[ag_in[jh][:]], outs=[ag_out[jh][:]])

            # ---- phase 2: symmetric blockwise cos-sim ----
            rowsum = const.tile([128, MT, NSLOT], F32)
            pos_acc = const.tile([128, MT], F32)
            corr_acc = const.tile([128, MT], F32)

            pid = nc.sync.partition_id()

            def dr_gemm(pg, m, rhs_ap, t):
                nc.tensor.matmul(
                    pg[:],
                    znT8[:, 2 * t:2 * t + 2, m * 128:(m + 1) * 128],
                    rhs_ap, start=(t == 0), stop=(t == 3), perf_mode=DR)

            def diag_to(dst_ap, pg, m, scale_exp):
                off = (m % 4) * 128
                jd = junkp.tile([128, 128], F32, tag="jd")
                nc.vector.tensor_tensor(jd[:], pg[:, off:off + 128],
                                        ident[:], ALU.mult)
                d = dtmpp.tile([128, 1], F32, tag="d")
                nc.vector.reduce_sum(d[:], jd[:], axis=mybir.AxisListType.X)
                if scale_exp:
                    nc.scalar.activation(dst_ap, d[:], AF.Exp, scale=EXP_SC)
                else:
                    nc.vector.tensor_scalar_mul(dst_ap, d[:], EXP_SC)

            with tc.tile_pool(name="rhs", bufs=8) as rhsp, \
                 tc.tile_pool(name="junk", bufs=4) as junkp, \
                 tc.tile_pool(name="jk8p", bufs=6) as jk8p, \
                 tc.tile_pool(name="dtmp", bufs=4) as dtmpp, \
                 tc.tile_pool(name="ps2", bufs=1, space="PSUM") as ps2:

                # local-block prelude (d=0, both col halves) — needs no AG
                for k in range(2):
                    for m in range(MT):
                        pg = ps2.tile([128, 512], F32, tag="pg", bufs=6,
                                      name=f"pgl{k}_{m}")
                        for t in range(4):
                            dr_gemm(pg, m,
                                    znT8[:, 2 * t:2 * t + 2,
                                         k * 512:(k + 1) * 512], t)
                        junk = junkp.tile([128, 512], BF16, tag="junk")
                        nc.scalar.activation(
                            junk[:], pg[:], AF.Exp, scale=EXP_SC,
                            accum_out=rowsum[:, m, k:k + 1])
                        if k == m // 4:
                            # self-similarity at compile-time position
                            diag_to(corr_acc[:, m:m + 1], pg, m,
                                    scale_exp=True)

                # remote blocks d = 1..4, dd-inner under shared stationary.
                # Exp tiles for dd=1..3 land in e5m2 slabs; their column
                # sums (one 4-instr DoubleRow ones-matmul per (k,dd)) ship
                # right after each k's dd<4 pass so the ReduceScatter can
                # start while the last pass (k=1, dd=4) still computes.
                rhs_t = {}
                for k in range(2):
                    for dd in range(1, 5):
                        row0 = ((pid + dd) % N_CORES) * D
                        rhs = rhsp.tile([128, KT, 512], FP8, tag="rhs",
                                        name=f"rhs{k}_{dd}")
                        src = ag_out[k][bass.ds(row0, D), :].rearrange(
                            "(kt p) j -> p kt j", p=128)
                        nc.sync.dma_start(rhs[:], src)
                        rhs_t[(k, dd)] = rhs

                def remote_pass(k, dds):
                    jk8 = {}
                    for dd in dds:
                        if dd < 4:
                            jk8[dd] = jk8p.tile([128, MT, 512], FP8E5,
                                                tag="jk8", name=f"jk8_{k}_{dd}")
                    for m in range(MT):
                        pgs = {dd: ps2.tile([128, 512], F32, tag="pg",
                                            bufs=6, name=f"pg{k}_{dd}_{m}")
                               for dd in dds}
                        for t in range(4):
                            for dd in dds:
                                dr_gemm(pgs[dd], m,
                                        rhs_t[(k, dd)][:, 2 * t:2 * t + 2, :],
                                        t)
                        for dd in dds:
                            slot = 2 + (dd - 1) * 2 + k
                            if dd < 4:
                                nc.scalar.activation(
                                    jk8[dd][:, m, :], pgs[dd][:], AF.Exp,
                                    scale=EXP_SC,
                                    accum_out=rowsum[:, m, slot:slot + 1])
                            else:
                                junk = junkp.tile([128, 512], BF16,
                                                  tag="junk")
                                nc.scalar.activation(
                                    junk[:], pgs[dd][:], AF.Exp,
                                    scale=EXP_SC,
                                    accum_out=rowsum[:, m, slot:slot + 1])
                                if k == m // 4:
                                    # positive-pair logit: block diagonal
                                    diag_to(pos_acc[:, m:m + 1], pgs[dd], m,
                                            scale_exp=False)
                    # ship this k's column sums to cores (c+dd)
                    for dd in dds:
                        if dd >= 4:
                            continue
                        cs = ps2.tile([1, 512], F32, tag="cs", bufs=1,
                                      name=f"cs{k}_{dd}")
                        for m in range(MT):
                            nc.tensor.matmul(
                                cs[:], ones_e5[:], jk8[dd][:, m, :],
                                start=(m == 0), stop=(m == MT - 1))
                        css = dtmpp.tile([1, 512], F32, tag="css",
                                         name=f"css{k}_{dd}")
                        nc.vector.tensor_copy(css[:], cs[:])
                        # static-offset publish; consumers read dynamically
                        # (dynamic DRAM writes go through a slow checked path)
                        nc.sync.dma_start(cs_in[k][dd - 1:dd, :], css[:])
                    if not any(dd < 4 for dd in dds):
                        return
                    if no_collective:
                        for c in range(N_CORES):
                            nc.sync.dma_start(
                                cs_out[k][c * 3:(c + 1) * 3, :], cs_in[k][:])
                    else:
                        nc.gpsimd.collective_compute(
                            "AllGather", ALU.bypass,
                            replica_groups=[list(range(N_CORES))],
                            ins=[cs_in[k][:]], outs=[cs_out[k][:]])

                remote_pass(0, [1, 2, 3])
                remote_pass(1, [1, 2, 3])
                # dd=4 passes run while the ReduceScatter (triggered by the
                # column-sum ships above) is in flight
                remote_pass(0, [4])
                remote_pass(1, [4])

                # ---- finale: lse, nll, partial sum (batched over m) ----
                rs = small.tile([128, MT], F32)
                nc.vector.reduce_sum(rs[:], rowsum[:],
                                     axis=mybir.AxisListType.X)
                # everything not gated on the exchange happens first
                nc.vector.tensor_tensor(rs[:], rs[:], corr_acc[:], ALU.subtract)
                rcv3 = small.tile([128, MT, 3], F32)
                for k in range(2):
                    for dd in range(1, 4):
                        row = (pid + (N_CORES - dd)) % N_CORES * 3 + (dd - 1)
                        nc.sync.dma_start(
                            rcv3[:, 4 * k:4 * (k + 1), dd - 1],
                            cs_out[k][bass.ds(row, 1), :]
                            .rearrange("one (m p) -> (one p) m", p=128))
                rcv = small.tile([128, MT], F32)
                nc.vector.reduce_sum(rcv[:], rcv3[:],
                                     axis=mybir.AxisListType.X)
                nc.vector.tensor_tensor(rs[:], rs[:], rcv[:], ALU.add)
                lse = small.tile([128, MT], F32)
                nc.scalar.activation(lse[:], rs[:], AF.Ln)
                nll = small.tile([128, MT], F32)
                nc.vector.tensor_tensor(nll[:], lse[:], pos_acc[:], ALU.subtract)
                pf = ps2.tile([1, MT], F32, tag="pf", bufs=1)
                nc.tensor.matmul(pf[:], ones_f[:], nll[:], start=True, stop=True)
                fs = small.tile([1, MT], F32)
                nc.vector.tensor_copy(fs[:], pf[:])
                nc.scalar.dma_start(out[:], fs[:])

    nc.compile()
    return nc


def kernel(x, w, b):
    if "nc" not in _cached:
        _cached["nc"] = _build()
    nc = _cached["nc"]
    x = np.ascontiguousarray(np.asarray(x, dtype=np.float32))
    w = np.ascontiguousarray(np.asarray(w, dtype=np.float32))
    b = np.ascontiguousarray(np.asarray(b, dtype=np.float32))
    in_maps = [{
        "xs": np.ascontiguousarray(x[c * R:(c + 1) * R]),
        "w": w, "b": b,
    } for c in range(N_CORES)]
    res = run_bass_kernel_spmd(nc, in_maps, list(range(N_CORES)))
    total = 0.0
    for c in range(N_CORES):
        total += float(res.results[c]["out"].astype(np.float64).sum())
    return np.float32(total / B)

